# revision 16
# baseline (speedup 1.0000x reference)
"""Trainium2 Bass kernel for nn_CSC1d (convolutional sparse coding, FISTA).

Reference computation (per batch element):
    L = lipschitz(D);  z0 = 0
    20x FISTA steps:
        grad = conv1d(convt1d(z, D) - x, D)
        w_new = relu(z - grad/L - lmbd/L)
        z_new = w_new + mu_i * (w_new - w_old)     (mu schedule is static)
    returns (convt1d(w_20, D), w_20)

Sharding: data-parallel over batch (16) across 8 cores -> 2 batch elements
per core; D-derived stationary matrices replicated; no collectives.

Device scheme (polyphase-16 layout, everything stays resident in SBUF):
  z-space tiles:  8 tiles per batch, rows = q*8+k' (q=phase 0..15, k'=atom%8),
                  cols = time-block n (16 samples per block)
  u-space tile:   rows = r'*8+c, cols = n
  convt1d  = 40 matmuls/batch: PSUM_u += St[l,h].T @ Z_h[:, n-l]
  r = u - x_poly (one fused DVE op, also evacuates PSUM)
  conv1d   = 40 matmuls/batch + 8 identity matmuls folding "+z" into PSUM:
             PSUM_g[h] += Sc[l,h].T @ r[:, n+l] + I.T @ Z_h
             (Sc carries the -1/L factor)
  w_new    = relu(gamma*PSUM - gamma*lmbd/L)  on ScalarE, PSUM -> SBUF
  z_new    = (w_new * s_i) - w_old            one scalar_tensor_tensor on DVE
             (scale bookkeeping: W stored pre-scaled by mu_{i+1})
  Matmuls run as float32r (full-rate fp32 mode on the PE).

kernel(x, D) -> (recon, z_hat), matching reference.py's return tuple.
"""

import numpy as np

B, C, T = 16, 8, 8192
K, KS = 64, 64
NCORES = 8
BLOC = B // NCORES      # batch elements per core
LMBD = 0.1
N_ITER = 20
PH = 16                 # polyphase factor
NB = T // PH            # 512 time blocks
TZ = T - KS + 1         # 8129 valid z length
NV = 509                # z blocks written (508 full + 1 partial)
ZPAD = 4                # left pad blocks on z/w tiles
ZP = ZPAD + NB          # 516 alloc cols for z/w tiles
P = 128
USE_F32R = True


def _mu_schedule():
    beta = 1.0
    mus = []
    for _ in range(N_ITER):
        beta_new = (1.0 + float(np.sqrt(1.0 + 4.0 * beta * beta))) / 2.0
        mus.append((beta - 1.0) / beta_new)
        beta = beta_new
    return mus


def _lipschitz(D):
    Fd = np.fft.fft(D.astype(np.float64), axis=2)
    L = (Fd.real ** 2 + Fd.imag ** 2).max(axis=2).sum()
    L = np.float32(L)
    return np.float32(1.0) if L == 0 else L


def _build_stationaries(D, L):
    """Stacked [81,128,128] fp32: 40 convt (St), 40 conv (Sc, scaled -1/L), identity."""
    D = D.astype(np.float32)
    l = np.arange(5)[:, None, None, None, None, None]
    h = np.arange(8)[None, :, None, None, None, None]
    a3 = np.arange(16)[None, None, :, None, None, None]
    a4 = np.arange(8)[None, None, None, :, None, None]
    a5 = np.arange(16)[None, None, None, None, :, None]
    a6 = np.arange(8)[None, None, None, None, None, :]
    # St[l,h, row=q*8+kp, col=rp*8+c] = D[8h+kp, c, 16l+rp-q]; dims (l,h,q,kp,rp,c)
    j = 16 * l + a5 - a3
    St = np.where((j >= 0) & (j < KS),
                  D[8 * h + a4, a6, np.clip(j, 0, KS - 1)], 0.0)
    St = St.reshape(5, 8, 128, 128).astype(np.float32)
    # Sc[l,h, row=i*8+c, col=q*8+kp] = -(1/L) D[8h+kp, c, 16l+i-q]; dims (l,h,i,c,q,kp)
    j2 = 16 * l + a3 - a5
    Sc = np.where((j2 >= 0) & (j2 < KS),
                  D[8 * h + a6, a4, np.clip(j2, 0, KS - 1)], 0.0)
    Sc = (Sc.reshape(5, 8, 128, 128) * (-1.0 / float(L))).astype(np.float32)
    ident = np.eye(128, dtype=np.float32)
    return np.concatenate(
        [St.reshape(40, 128, 128), Sc.reshape(40, 128, 128), ident[None]], axis=0)


NMM = 510            # conv matmul free dim (fp32r needs even N)
XW = NB + 3          # xp/r tile width (3 zero pad cols)


def _poly_x(xb):
    """x (C, T) -> [128 rows = rp*8+c, XW] (pad cols zero)."""
    out = np.zeros((P, XW), np.float32)
    out[:, :NB] = xb.reshape(C, NB, PH).transpose(2, 0, 1).reshape(P, NB)
    return out


_PROG_CACHE = {}
LAST_RESULT = None


def _build_program(L):
    import concourse.bacc as bacc
    import concourse.mybir as mybir
    import concourse.tile as tile

    f32 = mybir.dt.float32
    f32r = mybir.dt.float32r
    Relu = mybir.ActivationFunctionType.Relu
    Copy = mybir.ActivationFunctionType.Copy
    mult = mybir.AluOpType.mult
    sub = mybir.AluOpType.subtract

    mus = _mu_schedule()
    gammas = [mus[i + 1] if i + 1 < N_ITER else 1.0 for i in range(N_ITER)]
    L = float(L)

    nc = bacc.Bacc("TRN2", target_bir_lowering=False, debug=False)
    # register ACT bias constants (-gamma*lmbd/L per iteration) as const APs
    for i, v in enumerate(sorted({-g * LMBD / L for g in gammas})):
        t_ = nc.alloc_sbuf_tensor(f"constb{i}", [128, 1], f32)
        nc.gpsimd.memset(t_.ap(), v)
        nc.const_aps.aps[(f32, v)] = t_.ap()
    nc.all_engine_barrier()
    xp_d = nc.dram_tensor("xp", [BLOC, P, XW], f32r, kind="ExternalInput")
    ws_d = nc.dram_tensor("ws", [81, P, P], f32r, kind="ExternalInput")
    zp_d = nc.dram_tensor("zp", [BLOC, 8, P, NV], f32r, kind="ExternalOutput")
    up_d = nc.dram_tensor("up", [BLOC, P, NB], f32, kind="ExternalOutput")

    def mmcast(ap):
        if USE_F32R and ap.dtype != f32r:
            return ap.bitcast(f32r)
        return ap

    with tile.TileContext(nc) as tc:
      with tc.tile_pool(name="state", bufs=1) as state, \
           tc.tile_pool(name="psum", bufs=1, space="PSUM") as psum_pool:
        ws_t = state.tile([P, 81, P], f32r, name="ws_t", tag="ws_t")

        xp_t, Zt, Wt, rt, uf = [], [], [], [], []
        for b in range(BLOC):
            x_ = state.tile([P, XW], f32r, name=f"xp{b}", tag=f"xp{b}")
            nc.sync.dma_start(x_[:], xp_d[b])
            xp_t.append(x_)
            z_ = state.tile([P, 8, ZP], f32r, name=f"Z{b}", tag=f"Z{b}")
            Zt.append(z_)
            Wt.append([state.tile([P, 8, ZP], f32r, name=f"W{b}_{j}", tag=f"W{b}_{j}")
                       for j in range(2)])
            r_ = state.tile([P, XW], f32r, name=f"r{b}", tag=f"r{b}")
            rt.append(r_)
            uf.append(state.tile([P, NB], f32, name=f"uf{b}", tag=f"uf{b}"))
            # zero-init state (pads must stay zero forever)
            nc.vector.memset(Zt[b][:].bitcast(f32), 0.0)
            nc.gpsimd.memset(Wt[b][0][:].bitcast(f32), 0.0)
            nc.gpsimd.memset(Wt[b][1][:].bitcast(f32), 0.0)
            nc.vector.memset(rt[b][:, NB:XW].bitcast(f32), 0.0)
        # per-slice ws DMAs; conv stationaries (s=40..80) are needed first.
        # Round-robin across engines so the slices land in parallel queues.
        scratch = state.tile([P, NB], f32r, name="scratch", tag="scratch")
        nc.vector.memset(scratch[:].bitcast(f32), 0.0)
        dma_engines = [nc.sync, nc.gpsimd, nc.scalar]
        for k, s in enumerate(list(range(40, 81)) + list(range(0, 40))):
            dma_engines[k % 3].dma_start(ws_t[:, s, :], ws_d[s])

        if True:
            warm_ps = psum_pool.tile([P, NB], f32, tag="u", bufs=2, name="warm_ps")
            for _ in range(16):
                nc.tensor.matmul(warm_ps[:, :], mmcast(scratch[:, 0:P]),
                                 mmcast(scratch[:, 0:NB]), start=True, stop=True)
            wcur = [0] * BLOC
            for it in range(N_ITER):
                gam = gammas[it]
                # --- convt for both batches, one stationary load per (l,h) ---
                if it == 0:
                    rsrc = xp_t
                    act_scale = -gam    # PSUM holds -(z - grad/L) at it=0
                else:
                    u_ps = [psum_pool.tile([P, NB], f32, tag="u", bufs=2,
                                           name=f"u{it}_{b}") for b in range(BLOC)]
                    nmm = 0
                    for l5 in range(5):
                        for h in (list(range(1, 8)) + [0]):
                            for b in range(BLOC):
                                nc.tensor.matmul(
                                    u_ps[b][:, :],
                                    mmcast(ws_t[:, l5 * 8 + h, :]),
                                    mmcast(Zt[b][:, h, ZPAD - l5:ZPAD - l5 + NB]),
                                    start=(nmm < BLOC), stop=(nmm >= 78))
                                nmm += 1
                    for b in range(BLOC):
                        nc.vector.scalar_tensor_tensor(
                            out=rt[b][:, 0:NB], in0=u_ps[b][:, 0:NB], scalar=1.0,
                            in1=xp_t[b][:, 0:NB], op0=mult, op1=sub)
                    rsrc = rt
                    act_scale = gam
                wold = [Wt[b][wcur[b]] for b in range(BLOC)]
                wnew = [Wt[b][1 - wcur[b]] for b in range(BLOC)]
                # --- conv + identity per h-group, batches paired ---
                for h in (list(range(1, 8)) + [0]):
                    g_ps = psum_pool.tile([P, BLOC, NB], f32, tag="g", bufs=3,
                                          name=f"g{it}_{h}")
                    if it > 0:
                        for b in range(BLOC):
                            nc.tensor.matmul(
                                g_ps[:, b, 0:NMM], mmcast(ws_t[:, 80, :]),
                                mmcast(Zt[b][:, h, ZPAD:ZPAD + NMM]),
                                start=True, stop=False)
                    for l5 in range(5):
                        for b in range(BLOC):
                            nc.tensor.matmul(
                                g_ps[:, b, 0:NMM],
                                mmcast(ws_t[:, 40 + l5 * 8 + h, :]),
                                mmcast(rsrc[b][:, l5:l5 + NMM]),
                                start=(it == 0 and l5 == 0), stop=(l5 == 4))
                    for b in range(BLOC):
                        nc.scalar.activation(
                            out=wnew[b][:, h, ZPAD:ZPAD + 508], in_=g_ps[:, b, 0:508],
                            func=Relu, bias=-gam * LMBD / L, scale=act_scale)
                        # tail block n=508: only q=0 (rows 0..7) is a valid sample
                        nc.gpsimd.memset(
                            wnew[b][:, h, ZPAD + 508:ZPAD + 509].bitcast(f32), 0.0)
                        nc.scalar.activation(
                            out=wnew[b][0:8, h, ZPAD + 508:ZPAD + 509],
                            in_=g_ps[0:8, b, 508:509],
                            func=Relu, bias=-gam * LMBD / L, scale=act_scale)
                        if it + 1 < N_ITER:
                            s_i = (1.0 + mus[it]) / gam
                            nc.vector.scalar_tensor_tensor(
                                out=Zt[b][:, h, ZPAD:ZPAD + NV],
                                in0=wnew[b][:, h, ZPAD:ZPAD + NV], scalar=s_i,
                                in1=wold[b][:, h, ZPAD:ZPAD + NV], op0=mult, op1=sub)
                for b in range(BLOC):
                    wcur[b] = 1 - wcur[b]

            # final reconstruction convt on w_20 + outputs (batches paired)
            wfin = [Wt[b][wcur[b]] for b in range(BLOC)]
            for b in range(BLOC):
                nc.sync.dma_start(zp_d[b].rearrange("h p n -> p h n"),
                                  wfin[b][:, :, ZPAD:ZPAD + NV])
            u_ps = [psum_pool.tile([P, NB], f32, tag="u", bufs=2,
                                   name=f"uf_ps{b}") for b in range(BLOC)]
            nmm = 0
            for l5 in range(5):
                for h in range(8):
                    for b in range(BLOC):
                        nc.tensor.matmul(
                            u_ps[b][:, :], mmcast(ws_t[:, l5 * 8 + h, :]),
                            mmcast(wfin[b][:, h, ZPAD - l5:ZPAD - l5 + NB]),
                            start=(nmm < BLOC), stop=(nmm >= 78))
                        nmm += 1
            for b in range(BLOC):
                nc.scalar.activation(out=uf[b][:], in_=u_ps[b][:, :],
                                     func=Copy, bias=0.0, scale=1.0)
                nc.sync.dma_start(up_d[b], uf[b][:])

    nc.finalize()
    return nc


def _get_program(L):
    key = round(float(L), 6)
    if key not in _PROG_CACHE:
        _PROG_CACHE[key] = _build_program(L)
    return _PROG_CACHE[key]


def _trace_dir():
    import os
    d = '/root/problem/traces'
    if os.environ.get('BASS_TRACE') and os.path.isdir(d):
        return d
    return None


def kernel(x, D):
    from concourse.bass_utils import run_bass_kernel_spmd

    x = np.ascontiguousarray(np.asarray(x), dtype=np.float32)
    D = np.ascontiguousarray(np.asarray(D), dtype=np.float32)
    L = _lipschitz(D)
    ws = _build_stationaries(D, L)
    nc = _get_program(L)

    in_maps = []
    for i in range(NCORES):
        xp = np.stack([_poly_x(x[i * BLOC + b]) for b in range(BLOC)])
        in_maps.append({"xp": xp, "ws": ws})

    res = run_bass_kernel_spmd(nc, in_maps, list(range(NCORES)),
                               tmpdir=_trace_dir())
    global LAST_RESULT
    LAST_RESULT = res

    recon = np.empty((B, C, T), np.float32)
    z_hat = np.empty((B, K, TZ), np.float32)
    for i in range(NCORES):
        up = res.results[i]["up"]          # [BLOC, 128, 512]
        zp = res.results[i]["zp"]          # [BLOC, 8, 128, 509]
        for b in range(BLOC):
            gb = i * BLOC + b
            # recon[c, 16n+rp] = up[rp*8+c, n]
            recon[gb] = up[b].reshape(PH, C, NB).transpose(1, 2, 0).reshape(C, T)
            # z_hat[8h+kp, 16n+q] = zp[h, q*8+kp, n]
            zfull = zp[b].reshape(8, PH, 8, NV).transpose(0, 2, 3, 1).reshape(K, NV * PH)
            z_hat[gb] = zfull[:, :TZ]
    return recon, z_hat


# revision 18
# speedup vs baseline: 1.0042x; 1.0042x over previous
"""Trainium2 Bass kernel for nn_CSC1d (convolutional sparse coding, FISTA).

Reference computation (per batch element):
    L = lipschitz(D);  z0 = 0
    20x FISTA steps:
        grad = conv1d(convt1d(z, D) - x, D)
        w_new = relu(z - grad/L - lmbd/L)
        z_new = w_new + mu_i * (w_new - w_old)     (mu schedule is static)
    returns (convt1d(w_20, D), w_20)

Sharding: data-parallel over batch (16) across 8 cores -> 2 batch elements
per core; D-derived stationary matrices replicated; no collectives.

Device scheme (polyphase-16 layout, everything stays resident in SBUF):
  z-space tiles:  8 tiles per batch, rows = q*8+k' (q=phase 0..15, k'=atom%8),
                  cols = time-block n (16 samples per block)
  u-space tile:   rows = r'*8+c, cols = n
  convt1d  = 40 matmuls/batch: PSUM_u += St[l,h].T @ Z_h[:, n-l]
  r = u - x_poly (one fused DVE op, also evacuates PSUM)
  conv1d   = 40 matmuls/batch + 8 identity matmuls folding "+z" into PSUM:
             PSUM_g[h] += Sc[l,h].T @ r[:, n+l] + I.T @ Z_h
             (Sc carries the -1/L factor)
  w_new    = relu(gamma*PSUM - gamma*lmbd/L)  on ScalarE, PSUM -> SBUF
  z_new    = (w_new * s_i) - w_old            one scalar_tensor_tensor on DVE
             (scale bookkeeping: W stored pre-scaled by mu_{i+1})
  Matmuls run as float32r (full-rate fp32 mode on the PE).

kernel(x, D) -> (recon, z_hat), matching reference.py's return tuple.
"""

import numpy as np

B, C, T = 16, 8, 8192
K, KS = 64, 64
NCORES = 8
BLOC = B // NCORES      # batch elements per core
LMBD = 0.1
N_ITER = 20
PH = 16                 # polyphase factor
NB = T // PH            # 512 time blocks
TZ = T - KS + 1         # 8129 valid z length
NV = 509                # z blocks written (508 full + 1 partial)
ZPAD = 4                # left pad blocks on z/w tiles
ZP = ZPAD + NB          # 516 alloc cols for z/w tiles
P = 128
USE_F32R = True


def _mu_schedule():
    beta = 1.0
    mus = []
    for _ in range(N_ITER):
        beta_new = (1.0 + float(np.sqrt(1.0 + 4.0 * beta * beta))) / 2.0
        mus.append((beta - 1.0) / beta_new)
        beta = beta_new
    return mus


def _lipschitz(D):
    Fd = np.fft.fft(D.astype(np.float64), axis=2)
    L = (Fd.real ** 2 + Fd.imag ** 2).max(axis=2).sum()
    L = np.float32(L)
    return np.float32(1.0) if L == 0 else L


def _build_stationaries(D, L):
    """Stacked [81,128,128] fp32: 40 convt (St), 40 conv (Sc, scaled -1/L), identity."""
    D = D.astype(np.float32)
    l = np.arange(5)[:, None, None, None, None, None]
    h = np.arange(8)[None, :, None, None, None, None]
    a3 = np.arange(16)[None, None, :, None, None, None]
    a4 = np.arange(8)[None, None, None, :, None, None]
    a5 = np.arange(16)[None, None, None, None, :, None]
    a6 = np.arange(8)[None, None, None, None, None, :]
    # St[l,h, row=q*8+kp, col=rp*8+c] = D[8h+kp, c, 16l+rp-q]; dims (l,h,q,kp,rp,c)
    j = 16 * l + a5 - a3
    St = np.where((j >= 0) & (j < KS),
                  D[8 * h + a4, a6, np.clip(j, 0, KS - 1)], 0.0)
    St = St.reshape(5, 8, 128, 128).astype(np.float32)
    # Sc[l,h, row=i*8+c, col=q*8+kp] = -(1/L) D[8h+kp, c, 16l+i-q]; dims (l,h,i,c,q,kp)
    j2 = 16 * l + a3 - a5
    Sc = np.where((j2 >= 0) & (j2 < KS),
                  D[8 * h + a6, a4, np.clip(j2, 0, KS - 1)], 0.0)
    Sc = (Sc.reshape(5, 8, 128, 128) * (-1.0 / float(L))).astype(np.float32)
    ident = np.eye(128, dtype=np.float32)
    return np.concatenate(
        [St.reshape(40, 128, 128), Sc.reshape(40, 128, 128), ident[None]], axis=0)


NMM = 510            # conv matmul free dim (fp32r needs even N)
XW = NB + 3          # xp/r tile width (3 zero pad cols)


def _poly_x(xb):
    """x (C, T) -> [128 rows = rp*8+c, XW] (pad cols zero)."""
    out = np.zeros((P, XW), np.float32)
    out[:, :NB] = xb.reshape(C, NB, PH).transpose(2, 0, 1).reshape(P, NB)
    return out


_PROG_CACHE = {}
LAST_RESULT = None


def _build_program(L):
    import concourse.bacc as bacc
    import concourse.mybir as mybir
    import concourse.tile as tile

    f32 = mybir.dt.float32
    f32r = mybir.dt.float32r
    Relu = mybir.ActivationFunctionType.Relu
    Copy = mybir.ActivationFunctionType.Copy
    mult = mybir.AluOpType.mult
    sub = mybir.AluOpType.subtract

    mus = _mu_schedule()
    gammas = [mus[i + 1] if i + 1 < N_ITER else 1.0 for i in range(N_ITER)]
    L = float(L)

    nc = bacc.Bacc("TRN2", target_bir_lowering=False, debug=False)
    # register ACT bias constants (-gamma*lmbd/L per iteration) as const APs
    for i, v in enumerate(sorted({-g * LMBD / L for g in gammas})):
        t_ = nc.alloc_sbuf_tensor(f"constb{i}", [128, 1], f32)
        nc.gpsimd.memset(t_.ap(), v)
        nc.const_aps.aps[(f32, v)] = t_.ap()
    nc.all_engine_barrier()
    xp_d = nc.dram_tensor("xp", [BLOC, P, XW], f32r, kind="ExternalInput")
    ws_d = nc.dram_tensor("ws", [81, P, P], f32r, kind="ExternalInput")
    zp_d = nc.dram_tensor("zp", [BLOC, 8, P, NV], f32r, kind="ExternalOutput")
    up_d = nc.dram_tensor("up", [BLOC, P, NB], f32, kind="ExternalOutput")

    def mmcast(ap):
        if USE_F32R and ap.dtype != f32r:
            return ap.bitcast(f32r)
        return ap

    with tile.TileContext(nc) as tc:
      with tc.tile_pool(name="state", bufs=1) as state, \
           tc.tile_pool(name="psum", bufs=1, space="PSUM") as psum_pool:
        ws_t = state.tile([P, 81, P], f32r, name="ws_t", tag="ws_t")

        xp_t, Zt, Wt, rt, uf = [], [], [], [], []
        for b in range(BLOC):
            x_ = state.tile([P, XW], f32r, name=f"xp{b}", tag=f"xp{b}")
            nc.sync.dma_start(x_[:], xp_d[b])
            xp_t.append(x_)
            z_ = state.tile([P, 8, ZP], f32r, name=f"Z{b}", tag=f"Z{b}")
            Zt.append(z_)
            Wt.append([state.tile([P, 8, ZP], f32r, name=f"W{b}_{j}", tag=f"W{b}_{j}")
                       for j in range(2)])
            r_ = state.tile([P, XW], f32r, name=f"r{b}", tag=f"r{b}")
            rt.append(r_)
            uf.append(state.tile([P, NB], f32, name=f"uf{b}", tag=f"uf{b}"))
            # zero-init state (pads must stay zero forever)
            nc.vector.memset(Zt[b][:].bitcast(f32), 0.0)
            nc.gpsimd.memset(Wt[b][0][:].bitcast(f32), 0.0)
            nc.gpsimd.memset(Wt[b][1][:].bitcast(f32), 0.0)
            nc.vector.memset(rt[b][:, NB:XW].bitcast(f32), 0.0)
        # per-slice ws DMAs; conv stationaries (s=40..80) are needed first.
        # Round-robin across engines so the slices land in parallel queues.
        scratch = state.tile([P, NB], f32r, name="scratch", tag="scratch")
        nc.vector.memset(scratch[:].bitcast(f32), 0.0)
        dma_engines = [nc.sync, nc.gpsimd, nc.scalar]
        for k, s in enumerate(list(range(40, 81)) + list(range(0, 40))):
            dma_engines[k % 3].dma_start(ws_t[:, s, :], ws_d[s])

        if True:
            warm_ps = psum_pool.tile([P, NB], f32, tag="u", bufs=2, name="warm_ps")
            for _ in range(16):
                nc.tensor.matmul(warm_ps[:, :], mmcast(scratch[:, 0:P]),
                                 mmcast(scratch[:, 0:NB]), start=True, stop=True)
            wcur = [0] * BLOC
            for it in range(N_ITER):
                gam = gammas[it]
                # --- convt for both batches, one stationary load per (l,h) ---
                if it == 0:
                    rsrc = xp_t
                    act_scale = -gam    # PSUM holds -(z - grad/L) at it=0
                else:
                    u_ps = [psum_pool.tile([P, NB], f32, tag="u", bufs=2,
                                           name=f"u{it}_{b}") for b in range(BLOC)]
                    nmm = 0
                    for l5 in range(5):
                        for h in (list(range(1, 8)) + [0]):
                            for b in range(BLOC):
                                nc.tensor.matmul(
                                    u_ps[b][:, :],
                                    mmcast(ws_t[:, l5 * 8 + h, :]),
                                    mmcast(Zt[b][:, h, ZPAD - l5:ZPAD - l5 + NB]),
                                    start=(nmm < BLOC), stop=(nmm >= 78))
                                nmm += 1
                    for b in range(BLOC):
                        nc.vector.scalar_tensor_tensor(
                            out=rt[b][:, 0:NB], in0=u_ps[b][:, 0:NB], scalar=1.0,
                            in1=xp_t[b][:, 0:NB], op0=mult, op1=sub)
                    rsrc = rt
                    act_scale = gam
                wold = [Wt[b][wcur[b]] for b in range(BLOC)]
                wnew = [Wt[b][1 - wcur[b]] for b in range(BLOC)]
                # --- conv + identity per h-group, batches paired ---
                for h in (list(range(1, 8)) + [0]):
                    g_ps = psum_pool.tile([P, BLOC, NB], f32, tag="g", bufs=3,
                                          name=f"g{it}_{h}")
                    if it > 0:
                        for b in range(BLOC):
                            nc.tensor.matmul(
                                g_ps[:, b, 0:NMM], mmcast(ws_t[:, 80, :]),
                                mmcast(Zt[b][:, h, ZPAD:ZPAD + NMM]),
                                start=True, stop=False)
                    for l5 in range(5):
                        for b in range(BLOC):
                            nc.tensor.matmul(
                                g_ps[:, b, 0:NMM],
                                mmcast(ws_t[:, 40 + l5 * 8 + h, :]),
                                mmcast(rsrc[b][:, l5:l5 + NMM]),
                                start=(it == 0 and l5 == 0), stop=(l5 == 4))
                    for b in range(BLOC):
                        nc.scalar.activation(
                            out=wnew[b][:, h, ZPAD:ZPAD + 508], in_=g_ps[:, b, 0:508],
                            func=Relu, bias=-gam * LMBD / L, scale=act_scale)
                        # tail block n=508: only q=0 (rows 0..7) is a valid sample
                        nc.gpsimd.memset(
                            wnew[b][:, h, ZPAD + 508:ZPAD + 509].bitcast(f32), 0.0)
                        nc.scalar.activation(
                            out=wnew[b][0:8, h, ZPAD + 508:ZPAD + 509],
                            in_=g_ps[0:8, b, 508:509],
                            func=Relu, bias=-gam * LMBD / L, scale=act_scale)
                        if it + 1 < N_ITER:
                            s_i = (1.0 + mus[it]) / gam
                            nc.vector.scalar_tensor_tensor(
                                out=Zt[b][:, h, ZPAD:ZPAD + NV],
                                in0=wnew[b][:, h, ZPAD:ZPAD + NV], scalar=s_i,
                                in1=wold[b][:, h, ZPAD:ZPAD + NV], op0=mult, op1=sub)
                for b in range(BLOC):
                    wcur[b] = 1 - wcur[b]

            # final reconstruction convt on w_20 + outputs (batches paired)
            wfin = [Wt[b][wcur[b]] for b in range(BLOC)]
            for b in range(BLOC):
                nc.sync.dma_start(zp_d[b].rearrange("h p n -> p h n"),
                                  wfin[b][:, :, ZPAD:ZPAD + NV])
            u_ps = [psum_pool.tile([P, NB], f32, tag="u", bufs=2,
                                   name=f"uf_ps{b}") for b in range(BLOC)]
            nmm = 0
            for l5 in range(5):
                for h in range(8):
                    for b in range(BLOC):
                        nc.tensor.matmul(
                            u_ps[b][:, :], mmcast(ws_t[:, l5 * 8 + h, :]),
                            mmcast(wfin[b][:, h, ZPAD - l5:ZPAD - l5 + NB]),
                            start=(nmm < BLOC), stop=(nmm >= 78))
                        nmm += 1
            for b in range(BLOC):
                nc.scalar.activation(out=uf[b][:], in_=u_ps[b][:, :],
                                     func=Copy, bias=0.0, scale=1.0)
                nc.sync.dma_start(up_d[b], uf[b][:])

    nc.finalize()
    return nc


def _get_program(L):
    key = round(float(L), 6)
    if key not in _PROG_CACHE:
        _PROG_CACHE[key] = _build_program(L)
    return _PROG_CACHE[key]


def _trace_dir():
    import os
    d = '/root/problem/traces'
    if os.environ.get('BASS_TRACE') and os.path.isdir(d):
        return d
    return None


def kernel(x, D):
    from concourse.bass_utils import run_bass_kernel_spmd

    x = np.ascontiguousarray(np.asarray(x), dtype=np.float32)
    D = np.ascontiguousarray(np.asarray(D), dtype=np.float32)
    L = _lipschitz(D)
    ws = _build_stationaries(D, L)
    nc = _get_program(L)

    in_maps = []
    for i in range(NCORES):
        xp = np.stack([_poly_x(x[i * BLOC + b]) for b in range(BLOC)])
        in_maps.append({"xp": xp, "ws": ws})

    res = run_bass_kernel_spmd(nc, in_maps, list(range(NCORES)),
                               tmpdir=_trace_dir())
    global LAST_RESULT
    LAST_RESULT = res

    recon = np.empty((B, C, T), np.float32)
    z_hat = np.empty((B, K, TZ), np.float32)
    for i in range(NCORES):
        up = res.results[i]["up"]          # [BLOC, 128, 512]
        zp = res.results[i]["zp"]          # [BLOC, 8, 128, 509]
        for b in range(BLOC):
            gb = i * BLOC + b
            # recon[c, 16n+rp] = up[rp*8+c, n]
            recon[gb] = up[b].reshape(PH, C, NB).transpose(1, 2, 0).reshape(C, T)
            # z_hat[8h+kp, 16n+q] = zp[h, q*8+kp, n]
            zfull = zp[b].reshape(8, PH, 8, NV).transpose(0, 2, 3, 1).reshape(K, NV * PH)
            z_hat[gb] = zfull[:, :TZ]
    return recon, z_hat


# revision 19
# speedup vs baseline: 1.0077x; 1.0035x over previous
"""Trainium2 Bass kernel for nn_CSC1d (convolutional sparse coding, FISTA).

Reference computation (per batch element):
    L = lipschitz(D);  z0 = 0
    20x FISTA steps:
        grad = conv1d(convt1d(z, D) - x, D)
        w_new = relu(z - grad/L - lmbd/L)
        z_new = w_new + mu_i * (w_new - w_old)     (mu schedule is static)
    returns (convt1d(w_20, D), w_20)

Sharding: data-parallel over batch (16) across 8 cores -> 2 batch elements
per core; D-derived stationary matrices replicated; no collectives.

Device scheme (polyphase-16 layout, everything stays resident in SBUF):
  z-space tiles:  8 tiles per batch, rows = q*8+k' (q=phase 0..15, k'=atom%8),
                  cols = time-block n (16 samples per block)
  u-space tile:   rows = r'*8+c, cols = n
  convt1d  = 40 matmuls/batch: PSUM_u += St[l,h].T @ Z_h[:, n-l]
  r = u - x_poly (one fused DVE op, also evacuates PSUM)
  conv1d   = 40 matmuls/batch + 8 identity matmuls folding "+z" into PSUM:
             PSUM_g[h] += Sc[l,h].T @ r[:, n+l] + I.T @ Z_h
             (Sc carries the -1/L factor)
  w_new    = relu(gamma*PSUM - gamma*lmbd/L)  on ScalarE, PSUM -> SBUF
  z_new    = (w_new * s_i) - w_old            one scalar_tensor_tensor on DVE
             (scale bookkeeping: W stored pre-scaled by mu_{i+1})
  Matmuls run as float32r (full-rate fp32 mode on the PE).

kernel(x, D) -> (recon, z_hat), matching reference.py's return tuple.
"""

import numpy as np

B, C, T = 16, 8, 8192
K, KS = 64, 64
NCORES = 8
BLOC = B // NCORES      # batch elements per core
LMBD = 0.1
N_ITER = 20
PH = 16                 # polyphase factor
NB = T // PH            # 512 time blocks
TZ = T - KS + 1         # 8129 valid z length
NV = 509                # z blocks written (508 full + 1 partial)
ZPAD = 4                # left pad blocks on z/w tiles
ZP = ZPAD + NB          # 516 alloc cols for z/w tiles
P = 128
USE_F32R = True


def _mu_schedule():
    beta = 1.0
    mus = []
    for _ in range(N_ITER):
        beta_new = (1.0 + float(np.sqrt(1.0 + 4.0 * beta * beta))) / 2.0
        mus.append((beta - 1.0) / beta_new)
        beta = beta_new
    return mus


def _lipschitz(D):
    Fd = np.fft.fft(D.astype(np.float64), axis=2)
    L = (Fd.real ** 2 + Fd.imag ** 2).max(axis=2).sum()
    L = np.float32(L)
    return np.float32(1.0) if L == 0 else L


def _build_stationaries(D, L):
    """Stacked [81,128,128] fp32: 40 convt (St), 40 conv (Sc, scaled -1/L), identity."""
    D = D.astype(np.float32)
    l = np.arange(5)[:, None, None, None, None, None]
    h = np.arange(8)[None, :, None, None, None, None]
    a3 = np.arange(16)[None, None, :, None, None, None]
    a4 = np.arange(8)[None, None, None, :, None, None]
    a5 = np.arange(16)[None, None, None, None, :, None]
    a6 = np.arange(8)[None, None, None, None, None, :]
    # St[l,h, row=q*8+kp, col=rp*8+c] = D[8h+kp, c, 16l+rp-q]; dims (l,h,q,kp,rp,c)
    j = 16 * l + a5 - a3
    St = np.where((j >= 0) & (j < KS),
                  D[8 * h + a4, a6, np.clip(j, 0, KS - 1)], 0.0)
    St = St.reshape(5, 8, 128, 128).astype(np.float32)
    # Sc[l,h, row=i*8+c, col=q*8+kp] = -(1/L) D[8h+kp, c, 16l+i-q]; dims (l,h,i,c,q,kp)
    j2 = 16 * l + a3 - a5
    Sc = np.where((j2 >= 0) & (j2 < KS),
                  D[8 * h + a6, a4, np.clip(j2, 0, KS - 1)], 0.0)
    Sc = (Sc.reshape(5, 8, 128, 128) * (-1.0 / float(L))).astype(np.float32)
    ident = np.eye(128, dtype=np.float32)
    return np.concatenate(
        [St.reshape(40, 128, 128), Sc.reshape(40, 128, 128), ident[None]], axis=0)


NMM = 510            # conv matmul free dim (fp32r needs even N)
XW = NB + 3          # xp/r tile width (3 zero pad cols)


def _poly_x(xb):
    """x (C, T) -> [128 rows = rp*8+c, XW] (pad cols zero)."""
    out = np.zeros((P, XW), np.float32)
    out[:, :NB] = xb.reshape(C, NB, PH).transpose(2, 0, 1).reshape(P, NB)
    return out


_PROG_CACHE = {}
LAST_RESULT = None


def _build_program(L):
    import concourse.bacc as bacc
    import concourse.mybir as mybir
    import concourse.tile as tile

    f32 = mybir.dt.float32
    f32r = mybir.dt.float32r
    Relu = mybir.ActivationFunctionType.Relu
    Copy = mybir.ActivationFunctionType.Copy
    mult = mybir.AluOpType.mult
    sub = mybir.AluOpType.subtract

    mus = _mu_schedule()
    gammas = [mus[i + 1] if i + 1 < N_ITER else 1.0 for i in range(N_ITER)]
    L = float(L)

    nc = bacc.Bacc("TRN2", target_bir_lowering=False, debug=False)
    # register ACT bias constants (-gamma*lmbd/L per iteration) as const APs
    for i, v in enumerate(sorted({-g * LMBD / L for g in gammas})):
        t_ = nc.alloc_sbuf_tensor(f"constb{i}", [128, 1], f32)
        nc.gpsimd.memset(t_.ap(), v)
        nc.const_aps.aps[(f32, v)] = t_.ap()
    nc.all_engine_barrier()
    xp_d = nc.dram_tensor("xp", [BLOC, P, XW], f32r, kind="ExternalInput")
    ws_d = nc.dram_tensor("ws", [81, P, P], f32r, kind="ExternalInput")
    zp_d = nc.dram_tensor("zp", [BLOC, 8, P, NV], f32r, kind="ExternalOutput")
    up_d = nc.dram_tensor("up", [BLOC, P, NB], f32, kind="ExternalOutput")

    def mmcast(ap):
        if USE_F32R and ap.dtype != f32r:
            return ap.bitcast(f32r)
        return ap

    with tile.TileContext(nc) as tc:
      with tc.tile_pool(name="state", bufs=1) as state, \
           tc.tile_pool(name="psum", bufs=1, space="PSUM") as psum_pool:
        ws_t = state.tile([P, 81, P], f32r, name="ws_t", tag="ws_t")

        xp_t, Zt, Wt, rt, uf = [], [], [], [], []
        for b in range(BLOC):
            x_ = state.tile([P, XW], f32r, name=f"xp{b}", tag=f"xp{b}")
            nc.gpsimd.dma_start(x_[:], xp_d[b])
            xp_t.append(x_)
            z_ = state.tile([P, 8, ZP], f32r, name=f"Z{b}", tag=f"Z{b}")
            Zt.append(z_)
            Wt.append([state.tile([P, 8, ZP], f32r, name=f"W{b}_{j}", tag=f"W{b}_{j}")
                       for j in range(2)])
            r_ = state.tile([P, XW], f32r, name=f"r{b}", tag=f"r{b}")
            rt.append(r_)
            uf.append(state.tile([P, NB], f32, name=f"uf{b}", tag=f"uf{b}"))
            # zero-init state (pads must stay zero forever)
            nc.vector.memset(Zt[b][:].bitcast(f32), 0.0)
            nc.vector.memset(Wt[b][0][:].bitcast(f32), 0.0)
            nc.vector.memset(Wt[b][1][:].bitcast(f32), 0.0)
            nc.vector.memset(rt[b][:, NB:XW].bitcast(f32), 0.0)
        # per-slice ws DMAs; conv stationaries (s=40..80) are needed first.
        # Round-robin across engines so the slices land in parallel queues.
        scratch = state.tile([P, NB], f32r, name="scratch", tag="scratch")
        nc.vector.memset(scratch[:].bitcast(f32), 0.0)
        for s in list(range(40, 81)) + list(range(0, 40)):
            nc.sync.dma_start(ws_t[:, s, :], ws_d[s])

        if True:
            warm_ps = psum_pool.tile([P, NB], f32, tag="u", bufs=2, name="warm_ps")
            for _ in range(16):
                nc.tensor.matmul(warm_ps[:, :], mmcast(scratch[:, 0:P]),
                                 mmcast(scratch[:, 0:NB]), start=True, stop=True)
            wcur = [0] * BLOC
            for it in range(N_ITER):
                gam = gammas[it]
                # --- convt for both batches, one stationary load per (l,h) ---
                if it == 0:
                    rsrc = xp_t
                    act_scale = -gam    # PSUM holds -(z - grad/L) at it=0
                else:
                    u_ps = [psum_pool.tile([P, NB], f32, tag="u", bufs=2,
                                           name=f"u{it}_{b}") for b in range(BLOC)]
                    nmm = 0
                    for l5 in range(5):
                        for h in (list(range(1, 8)) + [0]):
                            for b in range(BLOC):
                                nc.tensor.matmul(
                                    u_ps[b][:, :],
                                    mmcast(ws_t[:, l5 * 8 + h, :]),
                                    mmcast(Zt[b][:, h, ZPAD - l5:ZPAD - l5 + NB]),
                                    start=(nmm < BLOC), stop=(nmm >= 78))
                                nmm += 1
                    for b in range(BLOC):
                        nc.vector.scalar_tensor_tensor(
                            out=rt[b][:, 0:NB], in0=u_ps[b][:, 0:NB], scalar=1.0,
                            in1=xp_t[b][:, 0:NB], op0=mult, op1=sub)
                    rsrc = rt
                    act_scale = gam
                wold = [Wt[b][wcur[b]] for b in range(BLOC)]
                wnew = [Wt[b][1 - wcur[b]] for b in range(BLOC)]
                # --- conv + identity per h-group, batches paired ---
                for h in (list(range(1, 8)) + [0]):
                    g_ps = psum_pool.tile([P, BLOC, NB], f32, tag="g", bufs=3,
                                          name=f"g{it}_{h}")
                    if it > 0:
                        for b in range(BLOC):
                            nc.tensor.matmul(
                                g_ps[:, b, 0:NMM], mmcast(ws_t[:, 80, :]),
                                mmcast(Zt[b][:, h, ZPAD:ZPAD + NMM]),
                                start=True, stop=False)
                    for l5 in range(5):
                        for b in range(BLOC):
                            nc.tensor.matmul(
                                g_ps[:, b, 0:NMM],
                                mmcast(ws_t[:, 40 + l5 * 8 + h, :]),
                                mmcast(rsrc[b][:, l5:l5 + NMM]),
                                start=(it == 0 and l5 == 0), stop=(l5 == 4))
                    for b in range(BLOC):
                        nc.scalar.activation(
                            out=wnew[b][:, h, ZPAD:ZPAD + 508], in_=g_ps[:, b, 0:508],
                            func=Relu, bias=-gam * LMBD / L, scale=act_scale)
                        # tail block n=508: only q=0 (rows 0..7) is a valid sample
                        nc.gpsimd.memset(
                            wnew[b][:, h, ZPAD + 508:ZPAD + 509].bitcast(f32), 0.0)
                        nc.scalar.activation(
                            out=wnew[b][0:8, h, ZPAD + 508:ZPAD + 509],
                            in_=g_ps[0:8, b, 508:509],
                            func=Relu, bias=-gam * LMBD / L, scale=act_scale)
                        if it + 1 < N_ITER:
                            s_i = (1.0 + mus[it]) / gam
                            nc.vector.scalar_tensor_tensor(
                                out=Zt[b][:, h, ZPAD:ZPAD + NV],
                                in0=wnew[b][:, h, ZPAD:ZPAD + NV], scalar=s_i,
                                in1=wold[b][:, h, ZPAD:ZPAD + NV], op0=mult, op1=sub)
                for b in range(BLOC):
                    wcur[b] = 1 - wcur[b]

            # final reconstruction convt on w_20 + outputs (batches paired)
            wfin = [Wt[b][wcur[b]] for b in range(BLOC)]
            for b in range(BLOC):
                nc.sync.dma_start(zp_d[b].rearrange("h p n -> p h n"),
                                  wfin[b][:, :, ZPAD:ZPAD + NV])
            u_ps = [psum_pool.tile([P, NB], f32, tag="u", bufs=2,
                                   name=f"uf_ps{b}") for b in range(BLOC)]
            nmm = 0
            for l5 in range(5):
                for h in range(8):
                    for b in range(BLOC):
                        nc.tensor.matmul(
                            u_ps[b][:, :], mmcast(ws_t[:, l5 * 8 + h, :]),
                            mmcast(wfin[b][:, h, ZPAD - l5:ZPAD - l5 + NB]),
                            start=(nmm < BLOC), stop=(nmm >= 78))
                        nmm += 1
            for b in range(BLOC):
                nc.scalar.activation(out=uf[b][:], in_=u_ps[b][:, :],
                                     func=Copy, bias=0.0, scale=1.0)
                nc.sync.dma_start(up_d[b], uf[b][:])

    nc.finalize()
    return nc


def _get_program(L):
    key = round(float(L), 6)
    if key not in _PROG_CACHE:
        _PROG_CACHE[key] = _build_program(L)
    return _PROG_CACHE[key]


def _trace_dir():
    import os
    d = '/root/problem/traces'
    if os.environ.get('BASS_TRACE') and os.path.isdir(d):
        return d
    return None


def kernel(x, D):
    from concourse.bass_utils import run_bass_kernel_spmd

    x = np.ascontiguousarray(np.asarray(x), dtype=np.float32)
    D = np.ascontiguousarray(np.asarray(D), dtype=np.float32)
    L = _lipschitz(D)
    ws = _build_stationaries(D, L)
    nc = _get_program(L)

    in_maps = []
    for i in range(NCORES):
        xp = np.stack([_poly_x(x[i * BLOC + b]) for b in range(BLOC)])
        in_maps.append({"xp": xp, "ws": ws})

    res = run_bass_kernel_spmd(nc, in_maps, list(range(NCORES)),
                               tmpdir=_trace_dir())
    global LAST_RESULT
    LAST_RESULT = res

    recon = np.empty((B, C, T), np.float32)
    z_hat = np.empty((B, K, TZ), np.float32)
    for i in range(NCORES):
        up = res.results[i]["up"]          # [BLOC, 128, 512]
        zp = res.results[i]["zp"]          # [BLOC, 8, 128, 509]
        for b in range(BLOC):
            gb = i * BLOC + b
            # recon[c, 16n+rp] = up[rp*8+c, n]
            recon[gb] = up[b].reshape(PH, C, NB).transpose(1, 2, 0).reshape(C, T)
            # z_hat[8h+kp, 16n+q] = zp[h, q*8+kp, n]
            zfull = zp[b].reshape(8, PH, 8, NV).transpose(0, 2, 3, 1).reshape(K, NV * PH)
            z_hat[gb] = zfull[:, :TZ]
    return recon, z_hat


# revision 21
# speedup vs baseline: 1.0100x; 1.0022x over previous
"""Trainium2 Bass kernel for nn_CSC1d (convolutional sparse coding, FISTA).

Reference computation (per batch element):
    L = lipschitz(D);  z0 = 0
    20x FISTA steps:
        grad = conv1d(convt1d(z, D) - x, D)
        w_new = relu(z - grad/L - lmbd/L)
        z_new = w_new + mu_i * (w_new - w_old)     (mu schedule is static)
    returns (convt1d(w_20, D), w_20)

Sharding: data-parallel over batch (16) across 8 cores -> 2 batch elements
per core; D-derived stationary matrices replicated; no collectives.

Device scheme (polyphase-16 layout, everything stays resident in SBUF):
  z-space tiles:  8 tiles per batch, rows = q*8+k' (q=phase 0..15, k'=atom%8),
                  cols = time-block n (16 samples per block)
  u-space tile:   rows = r'*8+c, cols = n
  convt1d  = 40 matmuls/batch: PSUM_u += St[l,h].T @ Z_h[:, n-l]
  r = u - x_poly (one fused DVE op, also evacuates PSUM)
  conv1d   = 40 matmuls/batch + 8 identity matmuls folding "+z" into PSUM:
             PSUM_g[h] += Sc[l,h].T @ r[:, n+l] + I.T @ Z_h
             (Sc carries the -1/L factor)
  w_new    = relu(gamma*PSUM - gamma*lmbd/L)  on ScalarE, PSUM -> SBUF
  z_new    = (w_new * s_i) - w_old            one scalar_tensor_tensor on DVE
             (scale bookkeeping: W stored pre-scaled by mu_{i+1})
  Matmuls run as float32r (full-rate fp32 mode on the PE).

kernel(x, D) -> (recon, z_hat), matching reference.py's return tuple.
"""

import numpy as np

B, C, T = 16, 8, 8192
K, KS = 64, 64
NCORES = 8
BLOC = B // NCORES      # batch elements per core
LMBD = 0.1
N_ITER = 20
PH = 16                 # polyphase factor
NB = T // PH            # 512 time blocks
TZ = T - KS + 1         # 8129 valid z length
NV = 509                # z blocks written (508 full + 1 partial)
ZPAD = 4                # left pad blocks on z/w tiles
ZP = ZPAD + NB          # 516 alloc cols for z/w tiles
P = 128
USE_F32R = True


def _mu_schedule():
    beta = 1.0
    mus = []
    for _ in range(N_ITER):
        beta_new = (1.0 + float(np.sqrt(1.0 + 4.0 * beta * beta))) / 2.0
        mus.append((beta - 1.0) / beta_new)
        beta = beta_new
    return mus


def _lipschitz(D):
    Fd = np.fft.fft(D.astype(np.float64), axis=2)
    L = (Fd.real ** 2 + Fd.imag ** 2).max(axis=2).sum()
    L = np.float32(L)
    return np.float32(1.0) if L == 0 else L


def _build_stationaries(D, L):
    """Stacked [81,128,128] fp32: 40 convt (St), 40 conv (Sc, scaled -1/L), identity."""
    D = D.astype(np.float32)
    l = np.arange(5)[:, None, None, None, None, None]
    h = np.arange(8)[None, :, None, None, None, None]
    a3 = np.arange(16)[None, None, :, None, None, None]
    a4 = np.arange(8)[None, None, None, :, None, None]
    a5 = np.arange(16)[None, None, None, None, :, None]
    a6 = np.arange(8)[None, None, None, None, None, :]
    # St[l,h, row=q*8+kp, col=rp*8+c] = D[8h+kp, c, 16l+rp-q]; dims (l,h,q,kp,rp,c)
    j = 16 * l + a5 - a3
    St = np.where((j >= 0) & (j < KS),
                  D[8 * h + a4, a6, np.clip(j, 0, KS - 1)], 0.0)
    St = St.reshape(5, 8, 128, 128).astype(np.float32)
    # Sc[l,h, row=i*8+c, col=q*8+kp] = -(1/L) D[8h+kp, c, 16l+i-q]; dims (l,h,i,c,q,kp)
    j2 = 16 * l + a3 - a5
    Sc = np.where((j2 >= 0) & (j2 < KS),
                  D[8 * h + a6, a4, np.clip(j2, 0, KS - 1)], 0.0)
    Sc = (Sc.reshape(5, 8, 128, 128) * (-1.0 / float(L))).astype(np.float32)
    ident = np.eye(128, dtype=np.float32)
    return np.concatenate(
        [St.reshape(40, 128, 128), Sc.reshape(40, 128, 128), ident[None]], axis=0)


NMM = 510            # conv matmul free dim (fp32r needs even N)
XW = NB + 3          # xp/r tile width (3 zero pad cols)


def _poly_x(xb):
    """x (C, T) -> [128 rows = rp*8+c, XW] (pad cols zero)."""
    out = np.zeros((P, XW), np.float32)
    out[:, :NB] = xb.reshape(C, NB, PH).transpose(2, 0, 1).reshape(P, NB)
    return out


_PROG_CACHE = {}
LAST_RESULT = None


def _build_program(L):
    import concourse.bacc as bacc
    import concourse.mybir as mybir
    import concourse.tile as tile

    f32 = mybir.dt.float32
    f32r = mybir.dt.float32r
    Relu = mybir.ActivationFunctionType.Relu
    Copy = mybir.ActivationFunctionType.Copy
    mult = mybir.AluOpType.mult
    sub = mybir.AluOpType.subtract

    mus = _mu_schedule()
    gammas = [mus[i + 1] if i + 1 < N_ITER else 1.0 for i in range(N_ITER)]
    L = float(L)

    nc = bacc.Bacc("TRN2", target_bir_lowering=False, debug=False)
    # register ACT bias constants (-gamma*lmbd/L per iteration) as const APs
    for i, v in enumerate(sorted({-g * LMBD / L for g in gammas})):
        t_ = nc.alloc_sbuf_tensor(f"constb{i}", [128, 1], f32)
        nc.gpsimd.memset(t_.ap(), v)
        nc.const_aps.aps[(f32, v)] = t_.ap()
    nc.all_engine_barrier()
    xp_d = nc.dram_tensor("xp", [BLOC, P, XW], f32r, kind="ExternalInput")
    ws_d = nc.dram_tensor("ws", [81, P, P], f32r, kind="ExternalInput")
    zp_d = nc.dram_tensor("zp", [BLOC, 8, P, NV], f32r, kind="ExternalOutput")
    up_d = nc.dram_tensor("up", [BLOC, P, NB], f32, kind="ExternalOutput")

    def mmcast(ap):
        if USE_F32R and ap.dtype != f32r:
            return ap.bitcast(f32r)
        return ap

    with tile.TileContext(nc) as tc:
      with tc.tile_pool(name="state", bufs=1) as state, \
           tc.tile_pool(name="psum", bufs=1, space="PSUM") as psum_pool:
        ws_t = state.tile([P, 81, P], f32r, name="ws_t", tag="ws_t")
        scratch = state.tile([P, NB], f32r, name="scratch", tag="scratch")
        nc.vector.memset(scratch[:].bitcast(f32), 0.0)

        xp_t, Zt, Wt, rt, uf = [], [], [], [], []
        for b in range(BLOC):
            x_ = state.tile([P, XW], f32r, name=f"xp{b}", tag=f"xp{b}")
            nc.gpsimd.dma_start(x_[:], xp_d[b])
            xp_t.append(x_)
            z_ = state.tile([P, 8, ZP], f32r, name=f"Z{b}", tag=f"Z{b}")
            Zt.append(z_)
            Wt.append([state.tile([P, 8, ZP], f32r, name=f"W{b}_{j}", tag=f"W{b}_{j}")
                       for j in range(2)])
            r_ = state.tile([P, XW], f32r, name=f"r{b}", tag=f"r{b}")
            rt.append(r_)
            uf.append(state.tile([P, NB], f32, name=f"uf{b}", tag=f"uf{b}"))
            # zero-init state (pads must stay zero forever)
            nc.vector.memset(Zt[b][:].bitcast(f32), 0.0)
            nc.gpsimd.memset(Wt[b][0][:].bitcast(f32), 0.0)
            nc.gpsimd.memset(Wt[b][1][:].bitcast(f32), 0.0)
            nc.vector.memset(rt[b][:, NB:XW].bitcast(f32), 0.0)
        # per-slice ws DMAs; conv stationaries (s=40..80) are needed first.
        # Round-robin across engines so the slices land in parallel queues.
        for s in list(range(40, 81)) + list(range(0, 40)):
            nc.sync.dma_start(ws_t[:, s, :], ws_d[s])

        if True:
            warm_ps = psum_pool.tile([P, NB], f32, tag="u", bufs=2, name="warm_ps")
            for _ in range(12):
                nc.tensor.matmul(warm_ps[:, :], mmcast(scratch[:, 0:P]),
                                 mmcast(scratch[:, 0:NB]), start=True, stop=True)
            wcur = [0] * BLOC
            for it in range(N_ITER):
                gam = gammas[it]
                # --- convt for both batches, one stationary load per (l,h) ---
                if it == 0:
                    rsrc = xp_t
                    act_scale = -gam    # PSUM holds -(z - grad/L) at it=0
                else:
                    u_ps = [psum_pool.tile([P, NB], f32, tag="u", bufs=2,
                                           name=f"u{it}_{b}") for b in range(BLOC)]
                    nmm = 0
                    for l5 in range(5):
                        for h in (list(range(1, 8)) + [0]):
                            for b in range(BLOC):
                                nc.tensor.matmul(
                                    u_ps[b][:, :],
                                    mmcast(ws_t[:, l5 * 8 + h, :]),
                                    mmcast(Zt[b][:, h, ZPAD - l5:ZPAD - l5 + NB]),
                                    start=(nmm < BLOC), stop=(nmm >= 78))
                                nmm += 1
                    for b in range(BLOC):
                        nc.vector.scalar_tensor_tensor(
                            out=rt[b][:, 0:NB], in0=u_ps[b][:, 0:NB], scalar=1.0,
                            in1=xp_t[b][:, 0:NB], op0=mult, op1=sub)
                    rsrc = rt
                    act_scale = gam
                wold = [Wt[b][wcur[b]] for b in range(BLOC)]
                wnew = [Wt[b][1 - wcur[b]] for b in range(BLOC)]
                # --- conv + identity per h-group, batches paired ---
                for h in (list(range(1, 8)) + [0]):
                    for b in range(BLOC):
                        g_ps = psum_pool.tile([P, NB], f32, tag="g", bufs=6,
                                              name=f"g{it}_{h}_{b}")
                        if it > 0:
                            nc.tensor.matmul(
                                g_ps[:, 0:NMM], mmcast(ws_t[:, 80, :]),
                                mmcast(Zt[b][:, h, ZPAD:ZPAD + NMM]),
                                start=True, stop=False)
                        for l5 in range(5):
                            nc.tensor.matmul(
                                g_ps[:, 0:NMM],
                                mmcast(ws_t[:, 40 + l5 * 8 + h, :]),
                                mmcast(rsrc[b][:, l5:l5 + NMM]),
                                start=(it == 0 and l5 == 0), stop=(l5 == 4))
                        nc.scalar.activation(
                            out=wnew[b][:, h, ZPAD:ZPAD + 508], in_=g_ps[:, 0:508],
                            func=Relu, bias=-gam * LMBD / L, scale=act_scale)
                        # tail block n=508: only q=0 (rows 0..7) is a valid sample
                        nc.gpsimd.memset(
                            wnew[b][:, h, ZPAD + 508:ZPAD + 509].bitcast(f32), 0.0)
                        nc.scalar.activation(
                            out=wnew[b][0:8, h, ZPAD + 508:ZPAD + 509],
                            in_=g_ps[0:8, 508:509],
                            func=Relu, bias=-gam * LMBD / L, scale=act_scale)
                        if it + 1 < N_ITER:
                            s_i = (1.0 + mus[it]) / gam
                            nc.vector.scalar_tensor_tensor(
                                out=Zt[b][:, h, ZPAD:ZPAD + NV],
                                in0=wnew[b][:, h, ZPAD:ZPAD + NV], scalar=s_i,
                                in1=wold[b][:, h, ZPAD:ZPAD + NV], op0=mult, op1=sub)
                for b in range(BLOC):
                    wcur[b] = 1 - wcur[b]

            # final reconstruction convt on w_20 + outputs (batches paired)
            wfin = [Wt[b][wcur[b]] for b in range(BLOC)]
            for b in range(BLOC):
                nc.sync.dma_start(zp_d[b].rearrange("h p n -> p h n"),
                                  wfin[b][:, :, ZPAD:ZPAD + NV])
            u_ps = [psum_pool.tile([P, NB], f32, tag="u", bufs=2,
                                   name=f"uf_ps{b}") for b in range(BLOC)]
            nmm = 0
            for l5 in range(5):
                for h in range(8):
                    for b in range(BLOC):
                        nc.tensor.matmul(
                            u_ps[b][:, :], mmcast(ws_t[:, l5 * 8 + h, :]),
                            mmcast(wfin[b][:, h, ZPAD - l5:ZPAD - l5 + NB]),
                            start=(nmm < BLOC), stop=(nmm >= 78))
                        nmm += 1
            for b in range(BLOC):
                nc.scalar.activation(out=uf[b][:], in_=u_ps[b][:, :],
                                     func=Copy, bias=0.0, scale=1.0)
                nc.sync.dma_start(up_d[b], uf[b][:])

    nc.finalize()
    return nc


def _get_program(L):
    key = round(float(L), 6)
    if key not in _PROG_CACHE:
        _PROG_CACHE[key] = _build_program(L)
    return _PROG_CACHE[key]


def _trace_dir():
    import os
    d = '/root/problem/traces'
    if os.environ.get('BASS_TRACE') and os.path.isdir(d):
        return d
    return None


def kernel(x, D):
    from concourse.bass_utils import run_bass_kernel_spmd

    x = np.ascontiguousarray(np.asarray(x), dtype=np.float32)
    D = np.ascontiguousarray(np.asarray(D), dtype=np.float32)
    L = _lipschitz(D)
    ws = _build_stationaries(D, L)
    nc = _get_program(L)

    in_maps = []
    for i in range(NCORES):
        xp = np.stack([_poly_x(x[i * BLOC + b]) for b in range(BLOC)])
        in_maps.append({"xp": xp, "ws": ws})

    res = run_bass_kernel_spmd(nc, in_maps, list(range(NCORES)),
                               tmpdir=_trace_dir())
    global LAST_RESULT
    LAST_RESULT = res

    recon = np.empty((B, C, T), np.float32)
    z_hat = np.empty((B, K, TZ), np.float32)
    for i in range(NCORES):
        up = res.results[i]["up"]          # [BLOC, 128, 512]
        zp = res.results[i]["zp"]          # [BLOC, 8, 128, 509]
        for b in range(BLOC):
            gb = i * BLOC + b
            # recon[c, 16n+rp] = up[rp*8+c, n]
            recon[gb] = up[b].reshape(PH, C, NB).transpose(1, 2, 0).reshape(C, T)
            # z_hat[8h+kp, 16n+q] = zp[h, q*8+kp, n]
            zfull = zp[b].reshape(8, PH, 8, NV).transpose(0, 2, 3, 1).reshape(K, NV * PH)
            z_hat[gb] = zfull[:, :TZ]
    return recon, z_hat
